# revision 44
# baseline (speedup 1.0000x reference)
"""Bidirectional cross-patch attention on 8 trn2 NeuronCores.

Sharding: data-parallel over B (4 batches x 2 cores), head-parallel within
each batch pair (6 heads per core). Each core computes q/k/v projections for
its heads, per-head masked attention, and a partial output projection; the
host sums the two partials per batch and adds the output bias.

Mask handling: allowed[i,j] = ctx_i ? ctx_j : 1. The additive -1e30 mask is
rank-1 (u_i * w_j with u=ctx, w=-1e30*(1-ctx)), so it is fused into the QK^T
matmul as a 65th contraction row. Logits are bounded (|s|~few), so softmax
needs no max subtraction: P = exp(scale*S_masked), denominator from an extra
ones-column in V.

Schedule: per head, QK tiles (S^T in PSUM, [128,1024]) ping-pong with ACT exp
(the pacing engine); AV accumulates per j-chunk right after its exp, and the
trailing AVs of each half are deferred into the next half's filler stream so
the PE never drains waiting for the last exps (keeps HAM at full clock).
Projections of the NEXT head-pair are emitted interleaved into the exp gaps;
the v projection for all 3 pairs runs as one N=390 pass (fillers in the
first head). Softmax denominators fold p-major to [128,8] via a DRAM
round-trip DMA, invert cheaply on DVE, and broadcast across 64 partitions
by another DMA, keeping the inversion off the ScalarE exp stream. Each
pair's first head writes its normalized output directly into the
out-projection layout; the second head repacks per-half. The last pair runs
heads/i-halves reversed so the kernel ends on a short skip-half with no
repack, with out-projection token tiles interleaved as fillers.
"""

from collections import deque

import numpy as np
import ml_dtypes

import concourse.bass as bass
import concourse.mybir as mybir
import concourse.tile as tile
from concourse.bass_utils import run_bass_kernel_spmd

BF16 = mybir.dt.bfloat16
F32 = mybir.dt.float32
bf16 = ml_dtypes.bfloat16

B, K, D, H, HD = 4, 2048, 768, 12, 64
HPC = 6        # heads per core
NPAIR = 3      # head pairs per core
NCHUNK = 6     # 768 / 128 contraction chunks
N_CORES = 8
NEG = -1e30
SCALE = 0.125  # 1/sqrt(HD)
NT = K // 128   # 16 token tiles of 128
NIB = K // 512  # 4 i-blocks of 512


def _split_multi_waits(nc, max_waits=1):
    """Walrus (CoreV3) rejects >1 sync-wait on one instruction; move extras
    onto no-op instructions inserted just before, preserving engine order."""
    for f in nc.m.functions:
        for bb in f.blocks:
            new_insts = []
            for inst in bb.instructions:
                si = inst.sync_info
                if si is not None and si.on_wait and len(si.on_wait) > max_waits:
                    waits = list(si.on_wait)
                    extra, keep = waits[:-max_waits], waits[-max_waits:]
                    for i in range(0, len(extra), max_waits):
                        chunk = extra[i:i + max_waits]
                        nop = mybir.InstNoOp(
                            name=f"waitsplit-{inst.name}-{i}",
                            engine=inst.engine,
                            sync_info=mybir.SyncInfo(on_wait=chunk, on_update=[]),
                        )
                        new_insts.append(nop)
                    si.on_wait = keep
                new_insts.append(inst)
            bb.instructions[:] = new_insts


def _build_nc(skip=True, split_waits=True):
    nc = bass.Bass()
    xT = nc.dram_tensor("xT", [NCHUNK, 128, K], BF16, kind="ExternalInput")
    wqT = nc.dram_tensor("wqT", [NCHUNK, 128, 384], BF16, kind="ExternalInput")
    wkT = nc.dram_tensor("wkT", [NCHUNK, 128, 384], BF16, kind="ExternalInput")
    wvT = nc.dram_tensor("wvT", [NCHUNK, 128, 390], BF16, kind="ExternalInput")
    woT = nc.dram_tensor("woT", [NPAIR, 128, D], BF16, kind="ExternalInput")
    bqv = nc.dram_tensor("bqv", [128, NPAIR], F32, kind="ExternalInput")
    bkv = nc.dram_tensor("bkv", [128, NPAIR], F32, kind="ExternalInput")
    bvv = nc.dram_tensor("bvv", [1, 390], BF16, kind="ExternalInput")
    uv = nc.dram_tensor("uv", [1, K], BF16, kind="ExternalInput")
    wv = nc.dram_tensor("wv", [1, K], BF16, kind="ExternalInput")
    # per-half staging rows for the 1/d fold + broadcast (engines cannot
    # move data across partitions, DMA can): d row -> DRAM -> [128,8] fold
    # for a cheap DVE reciprocal -> DRAM -> replicate over 64 partitions
    rbs = nc.dram_tensor("rbs", [NPAIR * 4, 1024], F32, kind="Internal")
    rbs2 = nc.dram_tensor("rbs2", [NPAIR * 4, 1024], F32, kind="Internal")
    out = nc.dram_tensor("out", [K, D], F32, kind="ExternalOutput")

    with tile.TileContext(nc) as tc:
        with (
            tc.tile_pool(name="const", bufs=1) as constp,
            tc.tile_pool(name="qpair", bufs=2) as qpp,
            tc.tile_pool(name="heads", bufs=2) as qkh,
            tc.tile_pool(name="ptp", bufs=11) as ptp,
            tc.tile_pool(name="yhp", bufs=2) as yhp,
            tc.tile_pool(name="ypk", bufs=1) as ypp,
            tc.tile_pool(name="small", bufs=2) as smp,
            tc.tile_pool(name="ost", bufs=2) as osp,
            tc.tile_pool(name="ps_a", bufs=2, space="PSUM") as ps_a,
            tc.tile_pool(name="ps_y", bufs=2, space="PSUM") as ps_y,
        ):
            # ---- warm the ACT table while the input DMAs run; Ln first so
            # the loader settles on natural_log_exp_and_others (has both Ln
            # and Exp -> no mid-kernel table switches)
            dummy = constp.tile([1, 1], F32, tag="dummy")
            nc.vector.memset(dummy, 1.0)
            nc.scalar.activation(dummy, dummy,
                                 mybir.ActivationFunctionType.Ln)
            nc.scalar.activation(dummy, dummy,
                                 mybir.ActivationFunctionType.Exp)

            # ---- load persistent operands (interleave x/wq/wk chunks across
            # two DMA queues so the first q-proj matmuls can start early)
            xts = [constp.tile([128, K], BF16, tag=f"xt{c}", name=f"xt{c}")
                   for c in range(NCHUNK)]
            wqs = [constp.tile([128, 384], BF16, tag=f"wq{c}", name=f"wq{c}")
                   for c in range(NCHUNK)]
            wks = [constp.tile([128, 384], BF16, tag=f"wk{c}", name=f"wk{c}")
                   for c in range(NCHUNK)]
            for c in range(NCHUNK):
                # alternate the six 512KB x chunks across both DMA queues:
                # serialized on one queue the last chunk lands ~14.6us in and
                # data-starves the first projection matmuls by ~5us
                (nc.gpsimd if c % 2 else nc.sync).dma_start(
                    out=xts[c], in_=xT[c])
                (nc.sync if c % 2 else nc.gpsimd).dma_start(
                    out=wqs[c], in_=wqT[c])
            bq_sb = constp.tile([128, NPAIR], F32, tag="bq")
            nc.gpsimd.dma_start(out=bq_sb, in_=bqv[:])
            for c in range(NCHUNK):
                (nc.sync if c % 2 else nc.gpsimd).dma_start(
                    out=wks[c], in_=wkT[c])
            bk_sb = constp.tile([128, NPAIR], F32, tag="bk")
            nc.sync.dma_start(out=bk_sb, in_=bkv[:])

            ones_sb = constp.tile([1, 128], BF16, tag="ones")
            nc.vector.memset(ones_sb, 1.0)
            # remaining weights ride the gpsimd queue behind wq/bq; they are
            # first needed a few tens of us in (v fillers / out-projection)
            wvs = [constp.tile([128, 390], BF16, tag=f"wv{c}", name=f"wv{c}")
                   for c in range(NCHUNK)]
            wos = [constp.tile([128, D], BF16, tag=f"wo{c}", name=f"wo{c}")
                   for c in range(NPAIR)]
            bv_sb = constp.tile([1, 390], BF16, tag="bv")
            # v for all pairs, [j-token partitions, tt, 6*65] with per-head
            # [v(64)|1] columns (ones column feeds the softmax denominator)
            vh_all = constp.tile([128, NT, 390], BF16, tag="vhall")
            for c in range(NCHUNK):
                nc.gpsimd.dma_start(out=wvs[c], in_=wvT[c])
            nc.gpsimd.dma_start(out=bv_sb, in_=bvv[:])
            for c in range(NPAIR):
                nc.gpsimd.dma_start(out=wos[c], in_=woT[c])

            ypk = [
                ypp.tile([128, K], BF16, tag=f"ypk{c}", name=f"ypk{c}")
                for c in range(NPAIR)
            ]

            def make_pair_setup(p):
                """Allocate pair-p tiles; return (state, qk_emitters)."""
                hsl = slice(p * 128, (p + 1) * 128)
                st = {
                    "qpair": qpp.tile([128, K], BF16, tag="qpair", name=f"qp{p}"),
                    "kpair": qpp.tile([128, K], BF16, tag="kpair", name=f"kp{p}"),
                    "qh": [qkh.tile([65, K], BF16, tag=f"qh{hh}", name=f"q{p}h{hh}")
                           for hh in range(2)],
                    "kh": [qkh.tile([65, K], BF16, tag=f"kh{hh}", name=f"k{p}h{hh}")
                           for hh in range(2)],
                }
                ems = []

                def qk_group(nm, ws, b_sb, tp, ib):
                    def em():
                        isl = slice(ib * 512, (ib + 1) * 512)
                        ps = ps_a.tile([128, 1024], F32, tag="a", name=f"pj{p}{nm}{ib}")
                        for c in range(NCHUNK):
                            nc.tensor.matmul(
                                ps[:, 0:512], ws[c][:, hsl], xts[c][:, isl],
                                start=(c == 0), stop=(c == NCHUNK - 1),
                            )
                        nc.vector.tensor_scalar_add(tp[:, isl], ps[:, 0:512],
                                                    b_sb[:, p:p + 1])
                    return em

                def repack(cs):
                    # per column-half, so the first QK (reading cols [0:1024))
                    # starts before the whole projection lands
                    def em():
                        for hh in range(2):
                            eng = nc.sync if hh == 0 else nc.gpsimd
                            eng.dma_start(
                                out=st["qh"][hh][0:64, cs],
                                in_=st["qpair"][hh * 64:(hh + 1) * 64, cs])
                            eng.dma_start(
                                out=st["kh"][hh][0:64, cs],
                                in_=st["kpair"][hh * 64:(hh + 1) * 64, cs])
                            if cs.start == 0:
                                eng.dma_start(out=st["qh"][hh][64:65, :],
                                              in_=uv[:])
                                eng.dma_start(out=st["kh"][hh][64:65, :],
                                              in_=wv[:])
                    return em

                for ib in range(2):
                    ems.append(qk_group("q", wqs, bq_sb, st["qpair"], ib))
                for ib in range(2):
                    ems.append(qk_group("k", wks, bk_sb, st["kpair"], ib))
                ems.append(repack(slice(0, 1024)))
                for ib in range(2, NIB):
                    ems.append(qk_group("q", wqs, bq_sb, st["qpair"], ib))
                for ib in range(2, NIB):
                    ems.append(qk_group("k", wks, bk_sb, st["kpair"], ib))
                ems.append(repack(slice(1024, 2048)))
                return st, ems

            # v projection: one N=390 pass per token tile covering all pairs
            def v_group(tt):
                def em():
                    tsl = slice(tt * 128, (tt + 1) * 128)
                    ps = ps_a.tile([128, 1024], F32, tag="a", name=f"pv{tt}")
                    for c in range(NCHUNK):
                        nc.tensor.matmul(
                            ps[:, 0:390], xts[c][:, tsl], wvs[c][:],
                            start=(c == 0), stop=False,
                        )
                    nc.tensor.matmul(
                        ps[:, 0:390], ones_sb[:, 0:128], bv_sb[:],
                        start=False, stop=True,
                    )
                    nc.vector.tensor_copy(vh_all[:, tt], ps[:, 0:390])
                return em

            # 5 j-chunks of lag keeps AV safely behind the exp stream (ACT
            # lags QK by <=2 chunks) while shortening each half's deferred
            # tail so the normalize chain launches two slots earlier
            AV_DELAY = 5
            NH = K // 1024  # 2 i-halves per head
            JC0 = 9  # with ctx-first sorted tokens: keys j >= JC0*128 are
            # non-context and queries i < 512 are context for every batch
            # (requires 512 <= n_ctx <= JC0*128, checked on the host), so
            # S^T blocks (jc >= JC0, i < 512) are exactly masked -> skipped.

            def emit_attention_half(p, hh, ih, st, fillers):
                """One head-half (1024 query columns): QK/exp/AV.

                AV matmuls are emitted AV_DELAY j-chunks late so that the
                previous half's normalize chain (which frees the y PSUM
                slots) never stalls the PE stream. Returns tail closures
                (reciprocal, broadcast, muls, repack) for the NEXT half's
                filler queue.
                """
                qt, kt = st["qh"][hh], st["kh"][hh]
                vsl = [slice(p * 130 + b * 65, p * 130 + (b + 1) * 65)
                       for b in range(2)]
                y_t = ps_y.tile([65, 1024], F32, tag="y", name=f"y{p}{hh}{ih}")
                y_tiles = [y_t[:, 0:512], y_t[:, 512:1024]]
                av_pending = deque()

                def av(jc):
                    def em():
                        if skip and ih == 0 and jc >= JC0:
                            # only query cols [896:1024) see non-context keys;
                            # continue the b=1 accumulation group (no start:
                            # has_written persists from jc < JC0)
                            nc.tensor.matmul(
                                y_tiles[1][:, 384:512],
                                vh_all[:, jc, vsl[hh]],
                                pts[jc][:, 896:1024],
                                start=False, stop=(jc == NT - 1),
                                skip_group_check=True,
                            )
                            return
                        stop_jc = (JC0 - 1) if (skip and ih == 0) else (NT - 1)
                        for b in range(2):
                            nc.tensor.matmul(
                                y_tiles[b], vh_all[:, jc, vsl[hh]],
                                pts[jc][:, b * 512:(b + 1) * 512],
                                start=(jc == 0), stop=(jc == stop_jc),
                                skip_group_check=skip and ih == 0,
                            )
                    return em

                pts = []
                for jc in range(NT):
                    skip_b0 = skip and ih == 0 and jc >= JC0
                    ptile = ptp.tile([128, 1024], BF16, tag="pt",
                                     name=f"pt{p}{hh}{ih}{jc}")
                    pts.append(ptile)
                    s_ps = ps_a.tile([128, 1024], F32, tag="a",
                                     name=f"s{p}{hh}{ih}{jc}")
                    if skip_b0:
                        # only query cols [896:1024) attend non-context keys
                        nc.tensor.matmul(
                            s_ps[:, 896:1024],
                            kt[:, jc * 128:(jc + 1) * 128],
                            qt[:, 896:1024],
                            start=True, stop=True,
                        )
                        nc.scalar.activation(
                            ptile[:, 896:1024], s_ps[:, 896:1024],
                            mybir.ActivationFunctionType.Exp, scale=SCALE,
                        )
                    else:
                        for b in range(2):
                            o = ih * 1024 + b * 512
                            nc.tensor.matmul(
                                s_ps[:, b * 512:(b + 1) * 512],
                                kt[:, jc * 128:(jc + 1) * 128],
                                qt[:, o:o + 512],
                                start=True, stop=True,
                            )
                        nc.scalar.activation(
                            ptile, s_ps,
                            mybir.ActivationFunctionType.Exp, scale=SCALE,
                        )
                    if fillers:
                        fillers.popleft()()
                        if len(fillers) > 9:
                            fillers.popleft()()
                    av_pending.append(av(jc))
                    if len(av_pending) > AV_DELAY:
                        av_pending.popleft()()
                # remaining AVs (last AV_DELAY+1 j-chunks) are NOT flushed
                # here: they join the tail and interleave with the next
                # half's QK stream, so the PE never drains waiting for the
                # trailing exps of this half.

                srow = smp.tile([1, 1024], F32, tag="srw", name=f"sr{p}{hh}{ih}")
                dcol = smp.tile([128, 8], F32, tag="dcl", name=f"dc{p}{hh}{ih}")
                rcol = smp.tile([128, 8], F32, tag="rcl", name=f"rc{p}{hh}{ih}")
                rb = smp.tile([64, 1024], F32, tag="rb", name=f"rb{p}{hh}{ih}")
                ridx = (p * 2 + hh) * 2 + ih

                def tail_recip():
                    # 1/d off the hot engines: copy the PSUM denominator row
                    # out, fold it p-major to [128,8] via a DRAM round trip
                    # (32B contiguous per partition), invert on DVE in ~0.2us,
                    # then unfold + replicate over 64 partitions by DMA.
                    # ScalarE keeps only the exp stream; the tail no longer
                    # queues behind it.
                    nc.vector.tensor_copy(srow, y_t[64:65, :])
                    nc.sync.dma_start(out=rbs[ridx:ridx + 1, :], in_=srow)
                    nc.gpsimd.dma_start(
                        out=dcol,
                        in_=rbs[ridx:ridx + 1, :].rearrange(
                            "a (p c) -> (a p) c", p=128))
                    nc.vector.reciprocal(rcol, dcol)
                    nc.sync.dma_start(
                        out=rbs2[ridx:ridx + 1, :].rearrange(
                            "a (p c) -> (a p) c", p=128),
                        in_=rcol)
                    nc.gpsimd.dma_start(
                        out=rb, in_=rbs2[ridx:ridx + 1, :].partition_broadcast(64))

                if hh == 0:
                    mul_out = ypk[p]
                    mul_rows = slice(0, 64)
                else:
                    mul_out = yhp.tile([64, 1024], BF16, tag="yh",
                                       name=f"yh{p}{ih}")
                    mul_rows = slice(0, 64)

                def tail_mul(b):
                    def em():
                        if hh == 0:
                            osl = slice(ih * 1024 + b * 512,
                                        ih * 1024 + (b + 1) * 512)
                        else:
                            osl = slice(b * 512, (b + 1) * 512)
                        nc.vector.tensor_mul(mul_out[mul_rows, osl],
                                             y_tiles[b][0:64, :],
                                             rb[:, b * 512:(b + 1) * 512])
                    return em

                tail = deque(av_pending)
                tail.extend([tail_recip] + [tail_mul(b) for b in range(2)])

                if hh == 1:
                    def tail_repack():
                        nc.gpsimd.dma_start(
                            out=ypk[p][64:128, ih * 1024:(ih + 1) * 1024],
                            in_=mul_out)
                    tail.append(tail_repack)
                return tail

            # ---- output projection emitter (partial over this core's 384
            # dy). Token tiles 0..7 only need ypk columns [0:1024), which are
            # complete after the last head's ih=0 tail -> they ride as
            # fillers inside the last half; the rest run at the end.
            def oproj(tt, on_scalar=False):
                def em():
                    tsl = slice(tt * 128, (tt + 1) * 128)
                    ot = osp.tile([128, D], F32, tag="ost", name=f"ot{tt}")
                    ps = ps_a.tile([128, 1024], F32, tag="a", name=f"po{tt}")
                    for oc, osz in ((0, 512), (1, 256)):
                        off = oc * 512
                        for c in range(NPAIR):
                            nc.tensor.matmul(
                                ps[:, off:off + osz],
                                ypk[c][:, tsl],
                                wos[c][:, off:off + osz],
                                start=(c == 0), stop=(c == NPAIR - 1),
                            )
                    eng = nc.sync if tt % 2 else nc.gpsimd
                    for oc, osz in ((0, 512), (1, 256)):
                        sl = slice(oc * 512, oc * 512 + osz)
                        if on_scalar:
                            # ScalarE is idle once the exps are done; store
                            # each half-tile as soon as its copy lands
                            nc.scalar.activation(
                                ot[:, sl], ps[:, sl],
                                mybir.ActivationFunctionType.Copy)
                            eng.dma_start(out=out[tsl, sl], in_=ot[:, sl])
                        else:
                            nc.vector.tensor_copy(ot[:, sl], ps[:, sl])
                    if not on_scalar:
                        eng.dma_start(out=out[tsl, :], in_=ot)
                return em

            # pair 0 q/k projection emitted inline; the all-pair v projection
            # rides as fillers inside the first head's QK loop
            st, ems = make_pair_setup(0)
            for em in ems:
                em()
            states = {0: st}
            tail = deque(v_group(tt) for tt in range(NT))
            # The last pair runs its heads and i-halves in reverse so the
            # kernel ends on the skip-half of the direct-write head: the
            # final normalize chain has the shortest ACT backlog and no
            # repack DMA, and out-proj token tiles 8..15 (whose ypk columns
            # complete one half earlier) interleave into the last half.
            for p in range(NPAIR):
                lastp = p == NPAIR - 1
                if not lastp:
                    states[p + 1], nxt = make_pair_setup(p + 1)
                    nxt = deque(nxt)
                else:
                    nxt = deque()
                hhs = (1, 0) if lastp else (0, 1)
                for hh in hhs:
                    ihs = (1, 0) if (lastp and hh == 0) else (0, 1)
                    for ih in ihs:
                        fillers = deque(tail)
                        if hh == hhs[0] and nxt:
                            # spread next-pair setup over this head's halves
                            take = ((len(nxt) + 1) // 2 if ih == ihs[0]
                                    else len(nxt))
                            for _ in range(take):
                                fillers.append(nxt.popleft())
                        if lastp and hh == 0 and ih == 0:
                            # out-proj for columns completed one half ago
                            for tt in range(8, NT):
                                fillers.append(oproj(tt))
                        tail = emit_attention_half(p, hh, ih, states[p],
                                                   fillers)
                        for em in fillers:
                            em()
            for em in tail:
                em()
            for tt in range(8):
                oproj(tt, on_scalar=True)()

    if split_waits:
        _split_multi_waits(nc)
    return nc


_NC = {}


def _get_nc(skip=True):
    if skip not in _NC:
        _NC[skip] = _build_nc(skip)
    return _NC[skip]


def sort_ok(is_context):
    n_ctx = np.asarray(is_context).sum(axis=1)
    return bool(np.all((n_ctx >= 896) & (n_ctx <= 128 * 9)))


def make_in_maps(x, is_context, Wq, bq, Wk, bk, Wv, bv, Wo, sort=True):
    """Host-side sharding/layout prep: per-core input dicts.

    With sort=True tokens are reordered context-first per batch (enables the
    masked-block skip); returns (in_maps, perms) where perms[b] un-sorts the
    output rows.
    """
    in_maps = []
    xTb = {}
    uvb = {}
    wvb = {}
    perms = []
    for b in range(B):
        ctx = is_context[b].astype(np.float32)
        if sort:
            perm = np.argsort(~is_context[b], kind="stable")
        else:
            perm = np.arange(K)
        perms.append(perm)
        xb = x[b][perm]
        ctx = ctx[perm]
        xTb[b] = np.ascontiguousarray(xb.T).astype(bf16).reshape(NCHUNK, 128, K)
        uvb[b] = ctx.reshape(1, K).astype(bf16)
        wvb[b] = (NEG * (1.0 - ctx)).reshape(1, K).astype(bf16)
    WoT = np.ascontiguousarray(Wo.T)  # [dy, dout]
    for core in range(N_CORES):
        b = core // 2
        half = core % 2
        sel = slice(384 * half, 384 * (half + 1))
        wvT_s = Wv[sel].T.astype(np.float32)  # [768, 384]
        wvTe = np.zeros((768, 390), np.float32)
        bve = np.zeros(390, np.float32)
        for pp in range(NPAIR):
            for hh in range(2):
                base = pp * 130 + hh * 65
                wcol = pp * 128 + hh * 64
                wvTe[:, base:base + 64] = wvT_s[:, wcol:wcol + 64]
                bve[base:base + 64] = bv[sel][wcol:wcol + 64]
                bve[base + 64] = 1.0
        in_maps.append({
            "xT": xTb[b],
            "wqT": np.ascontiguousarray(Wq[sel].T).astype(bf16).reshape(NCHUNK, 128, 384),
            "wkT": np.ascontiguousarray(Wk[sel].T).astype(bf16).reshape(NCHUNK, 128, 384),
            "wvT": wvTe.astype(bf16).reshape(NCHUNK, 128, 390),
            "woT": WoT[sel].astype(bf16).reshape(NPAIR, 128, D),
            "bqv": np.ascontiguousarray(bq[sel].reshape(NPAIR, 128).T).astype(np.float32),
            "bkv": np.ascontiguousarray(bk[sel].reshape(NPAIR, 128).T).astype(np.float32),
            "bvv": bve.astype(bf16).reshape(1, 390),
            "uv": uvb[b],
            "wv": wvb[b],
        })
    return in_maps, perms


def combine_results(results, bo, perms):
    out = np.zeros((B, K, D), np.float32)
    for b in range(B):
        out[b][perms[b]] = (
            results[2 * b]["out"] + results[2 * b + 1]["out"] + bo
        )
    return out


def kernel(x, is_context, Wq, bq, Wk, bk, Wv, bv, Wo, bo):
    x = np.asarray(x, np.float32)
    is_context = np.asarray(is_context)
    args = [np.asarray(a, np.float32) for a in (Wq, bq, Wk, bk, Wv, bv, Wo)]
    sort = sort_ok(is_context)
    nc = _get_nc(sort)
    in_maps, perms = make_in_maps(x, is_context, *args, sort=sort)
    res = run_bass_kernel_spmd(nc, in_maps, list(range(N_CORES)), trace=False)
    return combine_results(res.results, np.asarray(bo, np.float32), perms)


# revision 47
# speedup vs baseline: 1.0057x; 1.0057x over previous
"""Bidirectional cross-patch attention on 8 trn2 NeuronCores.

Sharding: data-parallel over B (4 batches x 2 cores), head-parallel within
each batch pair (6 heads per core). Each core computes q/k/v projections for
its heads, per-head masked attention, and a partial output projection; the
host sums the two partials per batch and adds the output bias.

Mask handling: allowed[i,j] = ctx_i ? ctx_j : 1. The additive -1e30 mask is
rank-1 (u_i * w_j with u=ctx, w=-1e30*(1-ctx)), so it is fused into the QK^T
matmul as a 65th contraction row. Logits are bounded (|s|~few), so softmax
needs no max subtraction: P = exp(scale*S_masked), denominator from an extra
ones-column in V.

Schedule: per head, QK tiles (S^T in PSUM, [128,1024]) ping-pong with ACT exp
(the pacing engine); AV accumulates per j-chunk right after its exp, and the
trailing AVs of each half are deferred into the next half's filler stream so
the PE never drains waiting for the last exps (keeps HAM at full clock).
Projections of the NEXT head-pair are emitted interleaved into the exp gaps;
the v projection for all 3 pairs runs as one N=390 pass (fillers in the
first head). Softmax denominators fold p-major to [128,8] via a DRAM
round-trip DMA, invert cheaply on DVE, and broadcast across 64 partitions
by another DMA, keeping the inversion off the ScalarE exp stream. Each
pair's first head writes its normalized output directly into the
out-projection layout; the second head repacks per-half. The last pair runs
heads/i-halves reversed so the kernel ends on a short skip-half with no
repack, with out-projection token tiles interleaved as fillers.
"""

from collections import deque

import numpy as np
import ml_dtypes

import concourse.bass as bass
import concourse.mybir as mybir
import concourse.tile as tile
from concourse.bass_utils import run_bass_kernel_spmd

BF16 = mybir.dt.bfloat16
F32 = mybir.dt.float32
bf16 = ml_dtypes.bfloat16

B, K, D, H, HD = 4, 2048, 768, 12, 64
HPC = 6        # heads per core
NPAIR = 3      # head pairs per core
NCHUNK = 6     # 768 / 128 contraction chunks
N_CORES = 8
NEG = -1e30
SCALE = 0.125  # 1/sqrt(HD)
NT = K // 128   # 16 token tiles of 128
NIB = K // 512  # 4 i-blocks of 512


def _split_multi_waits(nc, max_waits=1):
    """Walrus (CoreV3) rejects >1 sync-wait on one instruction; move extras
    onto no-op instructions inserted just before, preserving engine order."""
    for f in nc.m.functions:
        for bb in f.blocks:
            new_insts = []
            for inst in bb.instructions:
                si = inst.sync_info
                if si is not None and si.on_wait and len(si.on_wait) > max_waits:
                    waits = list(si.on_wait)
                    extra, keep = waits[:-max_waits], waits[-max_waits:]
                    for i in range(0, len(extra), max_waits):
                        chunk = extra[i:i + max_waits]
                        nop = mybir.InstNoOp(
                            name=f"waitsplit-{inst.name}-{i}",
                            engine=inst.engine,
                            sync_info=mybir.SyncInfo(on_wait=chunk, on_update=[]),
                        )
                        new_insts.append(nop)
                    si.on_wait = keep
                new_insts.append(inst)
            bb.instructions[:] = new_insts


def _build_nc(skip=True, split_waits=True):
    nc = bass.Bass()
    xT = nc.dram_tensor("xT", [NCHUNK, 128, K], BF16, kind="ExternalInput")
    wqT = nc.dram_tensor("wqT", [NCHUNK, 128, 384], BF16, kind="ExternalInput")
    wkT = nc.dram_tensor("wkT", [NCHUNK, 128, 384], BF16, kind="ExternalInput")
    wvT = nc.dram_tensor("wvT", [NCHUNK, 128, 390], BF16, kind="ExternalInput")
    woT = nc.dram_tensor("woT", [NPAIR, 128, D], BF16, kind="ExternalInput")
    bqv = nc.dram_tensor("bqv", [128, NPAIR], F32, kind="ExternalInput")
    bkv = nc.dram_tensor("bkv", [128, NPAIR], F32, kind="ExternalInput")
    bvv = nc.dram_tensor("bvv", [1, 390], BF16, kind="ExternalInput")
    uv = nc.dram_tensor("uv", [1, K], BF16, kind="ExternalInput")
    wv = nc.dram_tensor("wv", [1, K], BF16, kind="ExternalInput")
    # per-half staging rows for the 1/d fold + broadcast (engines cannot
    # move data across partitions, DMA can): d row -> DRAM -> [128,8] fold
    # for a cheap DVE reciprocal -> DRAM -> replicate over 64 partitions
    rbs = nc.dram_tensor("rbs", [NPAIR * 4, 1024], F32, kind="Internal")
    rbs2 = nc.dram_tensor("rbs2", [NPAIR * 4, 1024], F32, kind="Internal")
    out = nc.dram_tensor("out", [K, D], F32, kind="ExternalOutput")

    with tile.TileContext(nc) as tc:
        with (
            tc.tile_pool(name="const", bufs=1) as constp,
            tc.tile_pool(name="qpair", bufs=2) as qpp,
            tc.tile_pool(name="heads", bufs=2) as qkh,
            tc.tile_pool(name="ptp", bufs=13) as ptp,
            tc.tile_pool(name="yhp", bufs=2) as yhp,
            tc.tile_pool(name="ypk", bufs=1) as ypp,
            tc.tile_pool(name="small", bufs=2) as smp,
            tc.tile_pool(name="ost", bufs=2) as osp,
            tc.tile_pool(name="ps_a", bufs=2, space="PSUM") as ps_a,
            tc.tile_pool(name="ps_y", bufs=2, space="PSUM") as ps_y,
        ):
            # ---- warm the ACT table while the input DMAs run; Ln first so
            # the loader settles on natural_log_exp_and_others (has both Ln
            # and Exp -> no mid-kernel table switches)
            dummy = constp.tile([1, 1], F32, tag="dummy")
            nc.vector.memset(dummy, 1.0)
            nc.scalar.activation(dummy, dummy,
                                 mybir.ActivationFunctionType.Ln)
            nc.scalar.activation(dummy, dummy,
                                 mybir.ActivationFunctionType.Exp)

            # ---- load persistent operands (interleave x/wq/wk chunks across
            # two DMA queues so the first q-proj matmuls can start early)
            xts = [constp.tile([128, K], BF16, tag=f"xt{c}", name=f"xt{c}")
                   for c in range(NCHUNK)]
            wqs = [constp.tile([128, 384], BF16, tag=f"wq{c}", name=f"wq{c}")
                   for c in range(NCHUNK)]
            wks = [constp.tile([128, 384], BF16, tag=f"wk{c}", name=f"wk{c}")
                   for c in range(NCHUNK)]
            for c in range(NCHUNK):
                # alternate the six 512KB x chunks across both DMA queues:
                # serialized on one queue the last chunk lands ~14.6us in and
                # data-starves the first projection matmuls by ~5us
                (nc.gpsimd if c % 2 else nc.sync).dma_start(
                    out=xts[c], in_=xT[c])
                (nc.sync if c % 2 else nc.gpsimd).dma_start(
                    out=wqs[c], in_=wqT[c])
            bq_sb = constp.tile([128, NPAIR], F32, tag="bq")
            nc.gpsimd.dma_start(out=bq_sb, in_=bqv[:])
            for c in range(NCHUNK):
                (nc.sync if c % 2 else nc.gpsimd).dma_start(
                    out=wks[c], in_=wkT[c])
            bk_sb = constp.tile([128, NPAIR], F32, tag="bk")
            nc.sync.dma_start(out=bk_sb, in_=bkv[:])

            ones_sb = constp.tile([1, 128], BF16, tag="ones")
            nc.vector.memset(ones_sb, 1.0)
            # remaining weights ride the gpsimd queue behind wq/bq; they are
            # first needed a few tens of us in (v fillers / out-projection)
            wvs = [constp.tile([128, 390], BF16, tag=f"wv{c}", name=f"wv{c}")
                   for c in range(NCHUNK)]
            wos = [constp.tile([128, D], BF16, tag=f"wo{c}", name=f"wo{c}")
                   for c in range(NPAIR)]
            bv_sb = constp.tile([1, 390], BF16, tag="bv")
            # v for all pairs, [j-token partitions, tt, 6*65] with per-head
            # [v(64)|1] columns (ones column feeds the softmax denominator)
            vh_all = constp.tile([128, NT, 390], BF16, tag="vhall")
            for c in range(NCHUNK):
                nc.gpsimd.dma_start(out=wvs[c], in_=wvT[c])
            nc.gpsimd.dma_start(out=bv_sb, in_=bvv[:])
            for c in range(NPAIR):
                nc.gpsimd.dma_start(out=wos[c], in_=woT[c])

            ypk = [
                ypp.tile([128, K], BF16, tag=f"ypk{c}", name=f"ypk{c}")
                for c in range(NPAIR)
            ]

            def make_pair_setup(p):
                """Allocate pair-p tiles; return (state, qk_emitters)."""
                hsl = slice(p * 128, (p + 1) * 128)
                st = {
                    "qpair": qpp.tile([128, K], BF16, tag="qpair", name=f"qp{p}"),
                    "kpair": qpp.tile([128, K], BF16, tag="kpair", name=f"kp{p}"),
                    "qh": [qkh.tile([65, K], BF16, tag=f"qh{hh}", name=f"q{p}h{hh}")
                           for hh in range(2)],
                    "kh": [qkh.tile([65, K], BF16, tag=f"kh{hh}", name=f"k{p}h{hh}")
                           for hh in range(2)],
                }
                ems = []

                def qk_group(nm, ws, b_sb, tp, ib):
                    def em():
                        isl = slice(ib * 512, (ib + 1) * 512)
                        ps = ps_a.tile([128, 1024], F32, tag="a", name=f"pj{p}{nm}{ib}")
                        for c in range(NCHUNK):
                            nc.tensor.matmul(
                                ps[:, 0:512], ws[c][:, hsl], xts[c][:, isl],
                                start=(c == 0), stop=(c == NCHUNK - 1),
                            )
                        nc.vector.tensor_scalar_add(tp[:, isl], ps[:, 0:512],
                                                    b_sb[:, p:p + 1])
                    return em

                def repack(cs):
                    # per column-half, so the first QK (reading cols [0:1024))
                    # starts before the whole projection lands
                    def em():
                        for hh in range(2):
                            eng = nc.sync if hh == 0 else nc.gpsimd
                            eng.dma_start(
                                out=st["qh"][hh][0:64, cs],
                                in_=st["qpair"][hh * 64:(hh + 1) * 64, cs])
                            eng.dma_start(
                                out=st["kh"][hh][0:64, cs],
                                in_=st["kpair"][hh * 64:(hh + 1) * 64, cs])
                            if cs.start == 0:
                                eng.dma_start(out=st["qh"][hh][64:65, :],
                                              in_=uv[:])
                                eng.dma_start(out=st["kh"][hh][64:65, :],
                                              in_=wv[:])
                    return em

                for ib in range(2):
                    ems.append(qk_group("q", wqs, bq_sb, st["qpair"], ib))
                for ib in range(2):
                    ems.append(qk_group("k", wks, bk_sb, st["kpair"], ib))
                ems.append(repack(slice(0, 1024)))
                for ib in range(2, NIB):
                    ems.append(qk_group("q", wqs, bq_sb, st["qpair"], ib))
                for ib in range(2, NIB):
                    ems.append(qk_group("k", wks, bk_sb, st["kpair"], ib))
                ems.append(repack(slice(1024, 2048)))
                return st, ems

            # v projection: one N=390 pass per token tile covering all pairs
            def v_group(tt):
                def em():
                    tsl = slice(tt * 128, (tt + 1) * 128)
                    ps = ps_a.tile([128, 1024], F32, tag="a", name=f"pv{tt}")
                    for c in range(NCHUNK):
                        nc.tensor.matmul(
                            ps[:, 0:390], xts[c][:, tsl], wvs[c][:],
                            start=(c == 0), stop=False,
                        )
                    nc.tensor.matmul(
                        ps[:, 0:390], ones_sb[:, 0:128], bv_sb[:],
                        start=False, stop=True,
                    )
                    nc.vector.tensor_copy(vh_all[:, tt], ps[:, 0:390])
                return em

            # 9 j-chunks of AV lag: the ACT exp stream can run several
            # chunks behind the QK emission mid-half; extra slack keeps the
            # in-order PE queue from ever stalling at an AV matmul
            AV_DELAY = 9
            NH = K // 1024  # 2 i-halves per head
            JC0 = 9  # with ctx-first sorted tokens: keys j >= JC0*128 are
            # non-context and queries i < 512 are context for every batch
            # (requires 512 <= n_ctx <= JC0*128, checked on the host), so
            # S^T blocks (jc >= JC0, i < 512) are exactly masked -> skipped.

            def emit_attention_half(p, hh, ih, st, fillers):
                """One head-half (1024 query columns): QK/exp/AV.

                AV matmuls are emitted AV_DELAY j-chunks late so that the
                previous half's normalize chain (which frees the y PSUM
                slots) never stalls the PE stream. Returns tail closures
                (reciprocal, broadcast, muls, repack) for the NEXT half's
                filler queue.
                """
                qt, kt = st["qh"][hh], st["kh"][hh]
                vsl = [slice(p * 130 + b * 65, p * 130 + (b + 1) * 65)
                       for b in range(2)]
                y_t = ps_y.tile([65, 1024], F32, tag="y", name=f"y{p}{hh}{ih}")
                y_tiles = [y_t[:, 0:512], y_t[:, 512:1024]]
                av_pending = deque()

                def av(jc):
                    def em():
                        if skip and ih == 0 and jc >= JC0:
                            # only query cols [896:1024) see non-context keys;
                            # continue the b=1 accumulation group (no start:
                            # has_written persists from jc < JC0)
                            nc.tensor.matmul(
                                y_tiles[1][:, 384:512],
                                vh_all[:, jc, vsl[hh]],
                                pts[jc][:, 896:1024],
                                start=False, stop=(jc == NT - 1),
                                skip_group_check=True,
                            )
                            return
                        stop_jc = (JC0 - 1) if (skip and ih == 0) else (NT - 1)
                        for b in range(2):
                            nc.tensor.matmul(
                                y_tiles[b], vh_all[:, jc, vsl[hh]],
                                pts[jc][:, b * 512:(b + 1) * 512],
                                start=(jc == 0), stop=(jc == stop_jc),
                                skip_group_check=skip and ih == 0,
                            )
                    return em

                pts = []
                for jc in range(NT):
                    skip_b0 = skip and ih == 0 and jc >= JC0
                    ptile = ptp.tile([128, 1024], BF16, tag="pt",
                                     name=f"pt{p}{hh}{ih}{jc}")
                    pts.append(ptile)
                    s_ps = ps_a.tile([128, 1024], F32, tag="a",
                                     name=f"s{p}{hh}{ih}{jc}")
                    if skip_b0:
                        # only query cols [896:1024) attend non-context keys
                        nc.tensor.matmul(
                            s_ps[:, 896:1024],
                            kt[:, jc * 128:(jc + 1) * 128],
                            qt[:, 896:1024],
                            start=True, stop=True,
                        )
                        nc.scalar.activation(
                            ptile[:, 896:1024], s_ps[:, 896:1024],
                            mybir.ActivationFunctionType.Exp, scale=SCALE,
                        )
                    else:
                        for b in range(2):
                            o = ih * 1024 + b * 512
                            nc.tensor.matmul(
                                s_ps[:, b * 512:(b + 1) * 512],
                                kt[:, jc * 128:(jc + 1) * 128],
                                qt[:, o:o + 512],
                                start=True, stop=True,
                            )
                        nc.scalar.activation(
                            ptile, s_ps,
                            mybir.ActivationFunctionType.Exp, scale=SCALE,
                        )
                    if fillers:
                        fillers.popleft()()
                        if len(fillers) > 9:
                            fillers.popleft()()
                    av_pending.append(av(jc))
                    if len(av_pending) > AV_DELAY:
                        av_pending.popleft()()
                # remaining AVs (last AV_DELAY+1 j-chunks) are NOT flushed
                # here: they join the tail and interleave with the next
                # half's QK stream, so the PE never drains waiting for the
                # trailing exps of this half.

                srow = smp.tile([1, 1024], F32, tag="srw", name=f"sr{p}{hh}{ih}")
                dcol = smp.tile([128, 8], F32, tag="dcl", name=f"dc{p}{hh}{ih}")
                rcol = smp.tile([128, 8], F32, tag="rcl", name=f"rc{p}{hh}{ih}")
                rb = smp.tile([64, 1024], F32, tag="rb", name=f"rb{p}{hh}{ih}")
                ridx = (p * 2 + hh) * 2 + ih

                def tail_recip():
                    # 1/d off the hot engines: copy the PSUM denominator row
                    # out, fold it p-major to [128,8] via a DRAM round trip
                    # (32B contiguous per partition), invert on DVE in ~0.2us,
                    # then unfold + replicate over 64 partitions by DMA.
                    # ScalarE keeps only the exp stream; the tail no longer
                    # queues behind it.
                    nc.vector.tensor_copy(srow, y_t[64:65, :])
                    nc.sync.dma_start(out=rbs[ridx:ridx + 1, :], in_=srow)
                    nc.gpsimd.dma_start(
                        out=dcol,
                        in_=rbs[ridx:ridx + 1, :].rearrange(
                            "a (p c) -> (a p) c", p=128))
                    nc.vector.reciprocal(rcol, dcol)
                    nc.sync.dma_start(
                        out=rbs2[ridx:ridx + 1, :].rearrange(
                            "a (p c) -> (a p) c", p=128),
                        in_=rcol)
                    nc.gpsimd.dma_start(
                        out=rb, in_=rbs2[ridx:ridx + 1, :].partition_broadcast(64))

                if hh == 0:
                    mul_out = ypk[p]
                    mul_rows = slice(0, 64)
                else:
                    mul_out = yhp.tile([64, 1024], BF16, tag="yh",
                                       name=f"yh{p}{ih}")
                    mul_rows = slice(0, 64)

                def tail_mul(b):
                    def em():
                        if hh == 0:
                            osl = slice(ih * 1024 + b * 512,
                                        ih * 1024 + (b + 1) * 512)
                        else:
                            osl = slice(b * 512, (b + 1) * 512)
                        nc.vector.tensor_mul(mul_out[mul_rows, osl],
                                             y_tiles[b][0:64, :],
                                             rb[:, b * 512:(b + 1) * 512])
                    return em

                tail = deque(av_pending)
                tail.extend([tail_recip] + [tail_mul(b) for b in range(2)])

                if hh == 1:
                    def tail_repack():
                        nc.gpsimd.dma_start(
                            out=ypk[p][64:128, ih * 1024:(ih + 1) * 1024],
                            in_=mul_out)
                    tail.append(tail_repack)
                return tail

            # ---- output projection emitter (partial over this core's 384
            # dy). Token tiles 0..7 only need ypk columns [0:1024), which are
            # complete after the last head's ih=0 tail -> they ride as
            # fillers inside the last half; the rest run at the end.
            def oproj(tt, on_scalar=False):
                def em():
                    tsl = slice(tt * 128, (tt + 1) * 128)
                    ot = osp.tile([128, D], F32, tag="ost", name=f"ot{tt}")
                    ps = ps_a.tile([128, 1024], F32, tag="a", name=f"po{tt}")
                    for oc, osz in ((0, 512), (1, 256)):
                        off = oc * 512
                        for c in range(NPAIR):
                            nc.tensor.matmul(
                                ps[:, off:off + osz],
                                ypk[c][:, tsl],
                                wos[c][:, off:off + osz],
                                start=(c == 0), stop=(c == NPAIR - 1),
                            )
                    for oc, osz in ((0, 512), (1, 256)):
                        sl = slice(oc * 512, oc * 512 + osz)
                        if on_scalar:
                            # ScalarE is idle once the exps are done
                            nc.scalar.activation(
                                ot[:, sl], ps[:, sl],
                                mybir.ActivationFunctionType.Copy)
                        else:
                            nc.vector.tensor_copy(ot[:, sl], ps[:, sl])
                    (nc.sync if tt % 2 else nc.gpsimd).dma_start(
                        out=out[tsl, :], in_=ot)
                return em

            # pair 0 q/k projection emitted inline; the all-pair v projection
            # rides as fillers inside the first head's QK loop
            st, ems = make_pair_setup(0)
            for em in ems:
                em()
            states = {0: st}
            tail = deque(v_group(tt) for tt in range(NT))
            # The last pair runs its heads and i-halves in reverse so the
            # kernel ends on the skip-half of the direct-write head: the
            # final normalize chain has the shortest ACT backlog and no
            # repack DMA, and out-proj token tiles 8..15 (whose ypk columns
            # complete one half earlier) interleave into the last half.
            for p in range(NPAIR):
                lastp = p == NPAIR - 1
                if not lastp:
                    states[p + 1], nxt = make_pair_setup(p + 1)
                    nxt = deque(nxt)
                else:
                    nxt = deque()
                hhs = (1, 0) if lastp else (0, 1)
                for hh in hhs:
                    ihs = (1, 0) if (lastp and hh == 0) else (0, 1)
                    for ih in ihs:
                        fillers = deque(tail)
                        if hh == hhs[0] and nxt:
                            # spread next-pair setup over this head's halves
                            take = ((len(nxt) + 1) // 2 if ih == ihs[0]
                                    else len(nxt))
                            for _ in range(take):
                                fillers.append(nxt.popleft())
                        if lastp and hh == 0 and ih == 0:
                            # out-proj for columns completed one half ago
                            for tt in range(8, NT):
                                fillers.append(oproj(tt))
                        tail = emit_attention_half(p, hh, ih, states[p],
                                                   fillers)
                        for em in fillers:
                            em()
            for em in tail:
                em()
            for tt in range(8):
                oproj(tt, on_scalar=True)()

    if split_waits:
        _split_multi_waits(nc)
    return nc


_NC = {}


def _get_nc(skip=True):
    if skip not in _NC:
        _NC[skip] = _build_nc(skip)
    return _NC[skip]


def sort_ok(is_context):
    n_ctx = np.asarray(is_context).sum(axis=1)
    return bool(np.all((n_ctx >= 896) & (n_ctx <= 128 * 9)))


def make_in_maps(x, is_context, Wq, bq, Wk, bk, Wv, bv, Wo, sort=True):
    """Host-side sharding/layout prep: per-core input dicts.

    With sort=True tokens are reordered context-first per batch (enables the
    masked-block skip); returns (in_maps, perms) where perms[b] un-sorts the
    output rows.
    """
    in_maps = []
    xTb = {}
    uvb = {}
    wvb = {}
    perms = []
    for b in range(B):
        ctx = is_context[b].astype(np.float32)
        if sort:
            perm = np.argsort(~is_context[b], kind="stable")
        else:
            perm = np.arange(K)
        perms.append(perm)
        xb = x[b][perm]
        ctx = ctx[perm]
        xTb[b] = np.ascontiguousarray(xb.T).astype(bf16).reshape(NCHUNK, 128, K)
        uvb[b] = ctx.reshape(1, K).astype(bf16)
        wvb[b] = (NEG * (1.0 - ctx)).reshape(1, K).astype(bf16)
    WoT = np.ascontiguousarray(Wo.T)  # [dy, dout]
    for core in range(N_CORES):
        b = core // 2
        half = core % 2
        sel = slice(384 * half, 384 * (half + 1))
        wvT_s = Wv[sel].T.astype(np.float32)  # [768, 384]
        wvTe = np.zeros((768, 390), np.float32)
        bve = np.zeros(390, np.float32)
        for pp in range(NPAIR):
            for hh in range(2):
                base = pp * 130 + hh * 65
                wcol = pp * 128 + hh * 64
                wvTe[:, base:base + 64] = wvT_s[:, wcol:wcol + 64]
                bve[base:base + 64] = bv[sel][wcol:wcol + 64]
                bve[base + 64] = 1.0
        in_maps.append({
            "xT": xTb[b],
            "wqT": np.ascontiguousarray(Wq[sel].T).astype(bf16).reshape(NCHUNK, 128, 384),
            "wkT": np.ascontiguousarray(Wk[sel].T).astype(bf16).reshape(NCHUNK, 128, 384),
            "wvT": wvTe.astype(bf16).reshape(NCHUNK, 128, 390),
            "woT": WoT[sel].astype(bf16).reshape(NPAIR, 128, D),
            "bqv": np.ascontiguousarray(bq[sel].reshape(NPAIR, 128).T).astype(np.float32),
            "bkv": np.ascontiguousarray(bk[sel].reshape(NPAIR, 128).T).astype(np.float32),
            "bvv": bve.astype(bf16).reshape(1, 390),
            "uv": uvb[b],
            "wv": wvb[b],
        })
    return in_maps, perms


def combine_results(results, bo, perms):
    out = np.zeros((B, K, D), np.float32)
    for b in range(B):
        out[b][perms[b]] = (
            results[2 * b]["out"] + results[2 * b + 1]["out"] + bo
        )
    return out


def kernel(x, is_context, Wq, bq, Wk, bk, Wv, bv, Wo, bo):
    x = np.asarray(x, np.float32)
    is_context = np.asarray(is_context)
    args = [np.asarray(a, np.float32) for a in (Wq, bq, Wk, bk, Wv, bv, Wo)]
    sort = sort_ok(is_context)
    nc = _get_nc(sort)
    in_maps, perms = make_in_maps(x, is_context, *args, sort=sort)
    res = run_bass_kernel_spmd(nc, in_maps, list(range(N_CORES)), trace=False)
    return combine_results(res.results, np.asarray(bo, np.float32), perms)


# revision 48
# speedup vs baseline: 1.0108x; 1.0050x over previous
"""Bidirectional cross-patch attention on 8 trn2 NeuronCores.

Sharding: data-parallel over B (4 batches x 2 cores), head-parallel within
each batch pair (6 heads per core). Each core computes q/k/v projections for
its heads, per-head masked attention, and a partial output projection; the
host sums the two partials per batch and adds the output bias.

Mask handling: allowed[i,j] = ctx_i ? ctx_j : 1. The additive -1e30 mask is
rank-1 (u_i * w_j with u=ctx, w=-1e30*(1-ctx)), so it is fused into the QK^T
matmul as a 65th contraction row. Logits are bounded (|s|~few), so softmax
needs no max subtraction: P = exp(scale*S_masked), denominator from an extra
ones-column in V.

Schedule: per head, QK tiles (S^T in PSUM, [128,1024]) ping-pong with ACT exp
(the pacing engine); AV accumulates per j-chunk right after its exp, and the
trailing AVs of each half are deferred into the next half's filler stream so
the PE never drains waiting for the last exps (keeps HAM at full clock).
Projections of the NEXT head-pair are emitted interleaved into the exp gaps;
the v projection for all 3 pairs runs as one N=390 pass (fillers in the
first head). Softmax denominators fold p-major to [128,8] via a DRAM
round-trip DMA, invert cheaply on DVE, and broadcast across 64 partitions
by another DMA, keeping the inversion off the ScalarE exp stream. Each
pair's first head writes its normalized output directly into the
out-projection layout; the second head repacks per-half. The last pair runs
heads/i-halves reversed so the kernel ends on a short skip-half with no
repack, with out-projection token tiles interleaved as fillers.
"""

from collections import deque

import numpy as np
import ml_dtypes

import concourse.bass as bass
import concourse.mybir as mybir
import concourse.tile as tile
from concourse.bass_utils import run_bass_kernel_spmd

BF16 = mybir.dt.bfloat16
F32 = mybir.dt.float32
bf16 = ml_dtypes.bfloat16

B, K, D, H, HD = 4, 2048, 768, 12, 64
HPC = 6        # heads per core
NPAIR = 3      # head pairs per core
NCHUNK = 6     # 768 / 128 contraction chunks
N_CORES = 8
NEG = -1e30
SCALE = 0.125  # 1/sqrt(HD)
NT = K // 128   # 16 token tiles of 128
NIB = K // 512  # 4 i-blocks of 512


def _split_multi_waits(nc, max_waits=1):
    """Walrus (CoreV3) rejects >1 sync-wait on one instruction; move extras
    onto no-op instructions inserted just before, preserving engine order."""
    for f in nc.m.functions:
        for bb in f.blocks:
            new_insts = []
            for inst in bb.instructions:
                si = inst.sync_info
                if si is not None and si.on_wait and len(si.on_wait) > max_waits:
                    waits = list(si.on_wait)
                    extra, keep = waits[:-max_waits], waits[-max_waits:]
                    for i in range(0, len(extra), max_waits):
                        chunk = extra[i:i + max_waits]
                        nop = mybir.InstNoOp(
                            name=f"waitsplit-{inst.name}-{i}",
                            engine=inst.engine,
                            sync_info=mybir.SyncInfo(on_wait=chunk, on_update=[]),
                        )
                        new_insts.append(nop)
                    si.on_wait = keep
                new_insts.append(inst)
            bb.instructions[:] = new_insts


def _build_nc(skip=True, split_waits=True):
    nc = bass.Bass()
    xT = nc.dram_tensor("xT", [NCHUNK, 128, K], BF16, kind="ExternalInput")
    wqT = nc.dram_tensor("wqT", [NCHUNK, 128, 384], BF16, kind="ExternalInput")
    wkT = nc.dram_tensor("wkT", [NCHUNK, 128, 384], BF16, kind="ExternalInput")
    wvT = nc.dram_tensor("wvT", [NCHUNK, 128, 390], BF16, kind="ExternalInput")
    woT = nc.dram_tensor("woT", [NPAIR, 128, D], BF16, kind="ExternalInput")
    bqv = nc.dram_tensor("bqv", [128, NPAIR], F32, kind="ExternalInput")
    bkv = nc.dram_tensor("bkv", [128, NPAIR], F32, kind="ExternalInput")
    bvv = nc.dram_tensor("bvv", [1, 390], BF16, kind="ExternalInput")
    uv = nc.dram_tensor("uv", [1, K], BF16, kind="ExternalInput")
    wv = nc.dram_tensor("wv", [1, K], BF16, kind="ExternalInput")
    # per-half staging rows for the 1/d fold + broadcast (engines cannot
    # move data across partitions, DMA can): d row -> DRAM -> [128,8] fold
    # for a cheap DVE reciprocal -> DRAM -> replicate over 64 partitions
    rbs = nc.dram_tensor("rbs", [NPAIR * 4, 1024], F32, kind="Internal")
    rbs2 = nc.dram_tensor("rbs2", [NPAIR * 4, 1024], F32, kind="Internal")
    out = nc.dram_tensor("out", [K, D], F32, kind="ExternalOutput")

    with tile.TileContext(nc) as tc:
        with (
            tc.tile_pool(name="const", bufs=1) as constp,
            tc.tile_pool(name="qpair", bufs=2) as qpp,
            tc.tile_pool(name="heads", bufs=2) as qkh,
            tc.tile_pool(name="ptp", bufs=11) as ptp,
            tc.tile_pool(name="yhp", bufs=2) as yhp,
            tc.tile_pool(name="ypk", bufs=1) as ypp,
            tc.tile_pool(name="small", bufs=2) as smp,
            tc.tile_pool(name="ost", bufs=2) as osp,
            tc.tile_pool(name="ps_a", bufs=2, space="PSUM") as ps_a,
            tc.tile_pool(name="ps_y", bufs=2, space="PSUM") as ps_y,
        ):
            # ---- warm the ACT table while the input DMAs run; Ln first so
            # the loader settles on natural_log_exp_and_others (has both Ln
            # and Exp -> no mid-kernel table switches)
            dummy = constp.tile([1, 1], F32, tag="dummy")
            nc.vector.memset(dummy, 1.0)
            nc.scalar.activation(dummy, dummy,
                                 mybir.ActivationFunctionType.Ln)
            nc.scalar.activation(dummy, dummy,
                                 mybir.ActivationFunctionType.Exp)

            # ---- load persistent operands (interleave x/wq/wk chunks across
            # two DMA queues so the first q-proj matmuls can start early)
            xts = [constp.tile([128, K], BF16, tag=f"xt{c}", name=f"xt{c}")
                   for c in range(NCHUNK)]
            wqs = [constp.tile([128, 384], BF16, tag=f"wq{c}", name=f"wq{c}")
                   for c in range(NCHUNK)]
            wks = [constp.tile([128, 384], BF16, tag=f"wk{c}", name=f"wk{c}")
                   for c in range(NCHUNK)]
            for c in range(NCHUNK):
                # alternate the six 512KB x chunks across both DMA queues:
                # serialized on one queue the last chunk lands ~14.6us in and
                # data-starves the first projection matmuls by ~5us
                (nc.gpsimd if c % 2 else nc.sync).dma_start(
                    out=xts[c], in_=xT[c])
                (nc.sync if c % 2 else nc.gpsimd).dma_start(
                    out=wqs[c], in_=wqT[c])
            bq_sb = constp.tile([128, NPAIR], F32, tag="bq")
            nc.gpsimd.dma_start(out=bq_sb, in_=bqv[:])
            for c in range(NCHUNK):
                (nc.sync if c % 2 else nc.gpsimd).dma_start(
                    out=wks[c], in_=wkT[c])
            bk_sb = constp.tile([128, NPAIR], F32, tag="bk")
            nc.sync.dma_start(out=bk_sb, in_=bkv[:])

            ones_sb = constp.tile([1, 128], BF16, tag="ones")
            nc.vector.memset(ones_sb, 1.0)
            # remaining weights ride the gpsimd queue behind wq/bq; they are
            # first needed a few tens of us in (v fillers / out-projection)
            wvs = [constp.tile([128, 390], BF16, tag=f"wv{c}", name=f"wv{c}")
                   for c in range(NCHUNK)]
            wos = [constp.tile([128, D], BF16, tag=f"wo{c}", name=f"wo{c}")
                   for c in range(NPAIR)]
            bv_sb = constp.tile([1, 390], BF16, tag="bv")
            # v for all pairs, [j-token partitions, tt, 6*65] with per-head
            # [v(64)|1] columns (ones column feeds the softmax denominator)
            vh_all = constp.tile([128, NT, 390], BF16, tag="vhall")
            for c in range(NCHUNK):
                nc.gpsimd.dma_start(out=wvs[c], in_=wvT[c])
            nc.gpsimd.dma_start(out=bv_sb, in_=bvv[:])
            for c in range(NPAIR):
                nc.gpsimd.dma_start(out=wos[c], in_=woT[c])

            ypk = [
                ypp.tile([128, K], BF16, tag=f"ypk{c}", name=f"ypk{c}")
                for c in range(NPAIR)
            ]

            def make_pair_setup(p):
                """Allocate pair-p tiles; return (state, qk_emitters)."""
                hsl = slice(p * 128, (p + 1) * 128)
                st = {
                    "qpair": qpp.tile([128, K], BF16, tag="qpair", name=f"qp{p}"),
                    "kpair": qpp.tile([128, K], BF16, tag="kpair", name=f"kp{p}"),
                    "qh": [qkh.tile([65, K], BF16, tag=f"qh{hh}", name=f"q{p}h{hh}")
                           for hh in range(2)],
                    "kh": [qkh.tile([65, K], BF16, tag=f"kh{hh}", name=f"k{p}h{hh}")
                           for hh in range(2)],
                }
                ems = []

                def qk_group(nm, ws, b_sb, tp, ib):
                    def em():
                        isl = slice(ib * 512, (ib + 1) * 512)
                        ps = ps_a.tile([128, 1024], F32, tag="a", name=f"pj{p}{nm}{ib}")
                        for c in range(NCHUNK):
                            nc.tensor.matmul(
                                ps[:, 0:512], ws[c][:, hsl], xts[c][:, isl],
                                start=(c == 0), stop=(c == NCHUNK - 1),
                            )
                        nc.vector.tensor_scalar_add(tp[:, isl], ps[:, 0:512],
                                                    b_sb[:, p:p + 1])
                    return em

                def repack(cs):
                    # per column-half, so the first QK (reading cols [0:1024))
                    # starts before the whole projection lands
                    def em():
                        for hh in range(2):
                            eng = nc.sync if hh == 0 else nc.gpsimd
                            eng.dma_start(
                                out=st["qh"][hh][0:64, cs],
                                in_=st["qpair"][hh * 64:(hh + 1) * 64, cs])
                            eng.dma_start(
                                out=st["kh"][hh][0:64, cs],
                                in_=st["kpair"][hh * 64:(hh + 1) * 64, cs])
                            if cs.start == 0:
                                eng.dma_start(out=st["qh"][hh][64:65, :],
                                              in_=uv[:])
                                eng.dma_start(out=st["kh"][hh][64:65, :],
                                              in_=wv[:])
                    return em

                for ib in range(2):
                    ems.append(qk_group("q", wqs, bq_sb, st["qpair"], ib))
                for ib in range(2):
                    ems.append(qk_group("k", wks, bk_sb, st["kpair"], ib))
                ems.append(repack(slice(0, 1024)))
                for ib in range(2, NIB):
                    ems.append(qk_group("q", wqs, bq_sb, st["qpair"], ib))
                for ib in range(2, NIB):
                    ems.append(qk_group("k", wks, bk_sb, st["kpair"], ib))
                ems.append(repack(slice(1024, 2048)))
                return st, ems

            # v projection: one N=390 pass per token tile covering all pairs
            def v_group(tt):
                def em():
                    tsl = slice(tt * 128, (tt + 1) * 128)
                    ps = ps_a.tile([128, 1024], F32, tag="a", name=f"pv{tt}")
                    for c in range(NCHUNK):
                        nc.tensor.matmul(
                            ps[:, 0:390], xts[c][:, tsl], wvs[c][:],
                            start=(c == 0), stop=False,
                        )
                    nc.tensor.matmul(
                        ps[:, 0:390], ones_sb[:, 0:128], bv_sb[:],
                        start=False, stop=True,
                    )
                    nc.vector.tensor_copy(vh_all[:, tt], ps[:, 0:390])
                return em

            AV_DELAY = 7
            NH = K // 1024  # 2 i-halves per head
            JC0 = 9  # with ctx-first sorted tokens: keys j >= JC0*128 are
            # non-context and queries i < 512 are context for every batch
            # (requires 512 <= n_ctx <= JC0*128, checked on the host), so
            # S^T blocks (jc >= JC0, i < 512) are exactly masked -> skipped.

            def emit_attention_half(p, hh, ih, st, fillers):
                """One head-half (1024 query columns): QK/exp/AV.

                AV matmuls are emitted AV_DELAY j-chunks late so that the
                previous half's normalize chain (which frees the y PSUM
                slots) never stalls the PE stream. Returns tail closures
                (reciprocal, broadcast, muls, repack) for the NEXT half's
                filler queue.
                """
                qt, kt = st["qh"][hh], st["kh"][hh]
                vsl = [slice(p * 130 + b * 65, p * 130 + (b + 1) * 65)
                       for b in range(2)]
                y_t = ps_y.tile([65, 1024], F32, tag="y", name=f"y{p}{hh}{ih}")
                y_tiles = [y_t[:, 0:512], y_t[:, 512:1024]]
                av_pending = deque()

                def av(jc):
                    def em():
                        if skip and ih == 0 and jc >= JC0:
                            # only query cols [896:1024) see non-context keys;
                            # continue the b=1 accumulation group (no start:
                            # has_written persists from jc < JC0)
                            nc.tensor.matmul(
                                y_tiles[1][:, 384:512],
                                vh_all[:, jc, vsl[hh]],
                                pts[jc][:, 896:1024],
                                start=False, stop=(jc == NT - 1),
                                skip_group_check=True,
                            )
                            return
                        stop_jc = (JC0 - 1) if (skip and ih == 0) else (NT - 1)
                        for b in range(2):
                            nc.tensor.matmul(
                                y_tiles[b], vh_all[:, jc, vsl[hh]],
                                pts[jc][:, b * 512:(b + 1) * 512],
                                start=(jc == 0), stop=(jc == stop_jc),
                                skip_group_check=skip and ih == 0,
                            )
                    return em

                pts = []
                for jc in range(NT):
                    skip_b0 = skip and ih == 0 and jc >= JC0
                    ptile = ptp.tile([128, 1024], BF16, tag="pt",
                                     name=f"pt{p}{hh}{ih}{jc}")
                    pts.append(ptile)
                    s_ps = ps_a.tile([128, 1024], F32, tag="a",
                                     name=f"s{p}{hh}{ih}{jc}")
                    if skip_b0:
                        # only query cols [896:1024) attend non-context keys
                        nc.tensor.matmul(
                            s_ps[:, 896:1024],
                            kt[:, jc * 128:(jc + 1) * 128],
                            qt[:, 896:1024],
                            start=True, stop=True,
                        )
                        nc.scalar.activation(
                            ptile[:, 896:1024], s_ps[:, 896:1024],
                            mybir.ActivationFunctionType.Exp, scale=SCALE,
                        )
                    else:
                        for b in range(2):
                            o = ih * 1024 + b * 512
                            nc.tensor.matmul(
                                s_ps[:, b * 512:(b + 1) * 512],
                                kt[:, jc * 128:(jc + 1) * 128],
                                qt[:, o:o + 512],
                                start=True, stop=True,
                            )
                        nc.scalar.activation(
                            ptile, s_ps,
                            mybir.ActivationFunctionType.Exp, scale=SCALE,
                        )
                    if fillers:
                        fillers.popleft()()
                        if len(fillers) > 9:
                            fillers.popleft()()
                    av_pending.append(av(jc))
                    if len(av_pending) > AV_DELAY:
                        av_pending.popleft()()
                # remaining AVs (last AV_DELAY+1 j-chunks) are NOT flushed
                # here: they join the tail and interleave with the next
                # half's QK stream, so the PE never drains waiting for the
                # trailing exps of this half.

                srow = smp.tile([1, 1024], F32, tag="srw", name=f"sr{p}{hh}{ih}")
                dcol = smp.tile([128, 8], F32, tag="dcl", name=f"dc{p}{hh}{ih}")
                rcol = smp.tile([128, 8], F32, tag="rcl", name=f"rc{p}{hh}{ih}")
                rb = smp.tile([64, 1024], F32, tag="rb", name=f"rb{p}{hh}{ih}")
                ridx = (p * 2 + hh) * 2 + ih

                def tail_recip():
                    # 1/d off the hot engines: copy the PSUM denominator row
                    # out, fold it p-major to [128,8] via a DRAM round trip
                    # (32B contiguous per partition), invert on DVE in ~0.2us,
                    # then unfold + replicate over 64 partitions by DMA.
                    # ScalarE keeps only the exp stream; the tail no longer
                    # queues behind it.
                    nc.vector.tensor_copy(srow, y_t[64:65, :])
                    nc.sync.dma_start(out=rbs[ridx:ridx + 1, :], in_=srow)
                    nc.gpsimd.dma_start(
                        out=dcol,
                        in_=rbs[ridx:ridx + 1, :].rearrange(
                            "a (p c) -> (a p) c", p=128))
                    nc.vector.reciprocal(rcol, dcol)
                    nc.sync.dma_start(
                        out=rbs2[ridx:ridx + 1, :].rearrange(
                            "a (p c) -> (a p) c", p=128),
                        in_=rcol)
                    nc.gpsimd.dma_start(
                        out=rb, in_=rbs2[ridx:ridx + 1, :].partition_broadcast(64))

                if hh == 0:
                    mul_out = ypk[p]
                    mul_rows = slice(0, 64)
                else:
                    mul_out = yhp.tile([64, 1024], BF16, tag="yh",
                                       name=f"yh{p}{ih}")
                    mul_rows = slice(0, 64)

                def tail_mul(b):
                    def em():
                        if hh == 0:
                            osl = slice(ih * 1024 + b * 512,
                                        ih * 1024 + (b + 1) * 512)
                        else:
                            osl = slice(b * 512, (b + 1) * 512)
                        nc.vector.tensor_mul(mul_out[mul_rows, osl],
                                             y_tiles[b][0:64, :],
                                             rb[:, b * 512:(b + 1) * 512])
                    return em

                tail = deque(av_pending)
                tail.extend([tail_recip] + [tail_mul(b) for b in range(2)])

                if hh == 1:
                    def tail_repack():
                        nc.gpsimd.dma_start(
                            out=ypk[p][64:128, ih * 1024:(ih + 1) * 1024],
                            in_=mul_out)
                    tail.append(tail_repack)
                return tail

            # ---- output projection emitter (partial over this core's 384
            # dy). Token tiles 0..7 only need ypk columns [0:1024), which are
            # complete after the last head's ih=0 tail -> they ride as
            # fillers inside the last half; the rest run at the end.
            def oproj(tt, on_scalar=False):
                def em():
                    tsl = slice(tt * 128, (tt + 1) * 128)
                    ot = osp.tile([128, D], F32, tag="ost", name=f"ot{tt}")
                    ps = ps_a.tile([128, 1024], F32, tag="a", name=f"po{tt}")
                    for oc, osz in ((0, 512), (1, 256)):
                        off = oc * 512
                        for c in range(NPAIR):
                            nc.tensor.matmul(
                                ps[:, off:off + osz],
                                ypk[c][:, tsl],
                                wos[c][:, off:off + osz],
                                start=(c == 0), stop=(c == NPAIR - 1),
                            )
                    for oc, osz in ((0, 512), (1, 256)):
                        sl = slice(oc * 512, oc * 512 + osz)
                        if on_scalar:
                            # ScalarE is idle once the exps are done
                            nc.scalar.activation(
                                ot[:, sl], ps[:, sl],
                                mybir.ActivationFunctionType.Copy)
                        else:
                            nc.vector.tensor_copy(ot[:, sl], ps[:, sl])
                    (nc.sync if tt % 2 else nc.gpsimd).dma_start(
                        out=out[tsl, :], in_=ot)
                return em

            # pair 0 q/k projection emitted inline; the all-pair v projection
            # rides as fillers inside the first head's QK loop
            st, ems = make_pair_setup(0)
            for em in ems:
                em()
            states = {0: st}
            tail = deque(v_group(tt) for tt in range(NT))
            # The last pair runs its heads and i-halves in reverse so the
            # kernel ends on the skip-half of the direct-write head: the
            # final normalize chain has the shortest ACT backlog and no
            # repack DMA, and out-proj token tiles 8..15 (whose ypk columns
            # complete one half earlier) interleave into the last half.
            for p in range(NPAIR):
                lastp = p == NPAIR - 1
                if not lastp:
                    states[p + 1], nxt = make_pair_setup(p + 1)
                    nxt = deque(nxt)
                else:
                    nxt = deque()
                hhs = (1, 0) if lastp else (0, 1)
                for hh in hhs:
                    ihs = (1, 0) if (lastp and hh == 0) else (0, 1)
                    for ih in ihs:
                        fillers = deque(tail)
                        if hh == hhs[0] and nxt:
                            # spread next-pair setup over this head's halves
                            take = ((len(nxt) + 1) // 2 if ih == ihs[0]
                                    else len(nxt))
                            for _ in range(take):
                                fillers.append(nxt.popleft())
                        if lastp and hh == 0 and ih == 0:
                            # out-proj for columns completed one half ago
                            for tt in range(8, NT):
                                fillers.append(oproj(tt))
                        tail = emit_attention_half(p, hh, ih, states[p],
                                                   fillers)
                        for em in fillers:
                            em()
            for em in tail:
                em()
            for tt in range(8):
                oproj(tt, on_scalar=True)()

    if split_waits:
        _split_multi_waits(nc)
    return nc


_NC = {}


def _get_nc(skip=True):
    if skip not in _NC:
        _NC[skip] = _build_nc(skip)
    return _NC[skip]


def sort_ok(is_context):
    n_ctx = np.asarray(is_context).sum(axis=1)
    return bool(np.all((n_ctx >= 896) & (n_ctx <= 128 * 9)))


def make_in_maps(x, is_context, Wq, bq, Wk, bk, Wv, bv, Wo, sort=True):
    """Host-side sharding/layout prep: per-core input dicts.

    With sort=True tokens are reordered context-first per batch (enables the
    masked-block skip); returns (in_maps, perms) where perms[b] un-sorts the
    output rows.
    """
    in_maps = []
    xTb = {}
    uvb = {}
    wvb = {}
    perms = []
    for b in range(B):
        ctx = is_context[b].astype(np.float32)
        if sort:
            perm = np.argsort(~is_context[b], kind="stable")
        else:
            perm = np.arange(K)
        perms.append(perm)
        xb = x[b][perm]
        ctx = ctx[perm]
        xTb[b] = np.ascontiguousarray(xb.T).astype(bf16).reshape(NCHUNK, 128, K)
        uvb[b] = ctx.reshape(1, K).astype(bf16)
        wvb[b] = (NEG * (1.0 - ctx)).reshape(1, K).astype(bf16)
    WoT = np.ascontiguousarray(Wo.T)  # [dy, dout]
    for core in range(N_CORES):
        b = core // 2
        half = core % 2
        sel = slice(384 * half, 384 * (half + 1))
        wvT_s = Wv[sel].T.astype(np.float32)  # [768, 384]
        wvTe = np.zeros((768, 390), np.float32)
        bve = np.zeros(390, np.float32)
        for pp in range(NPAIR):
            for hh in range(2):
                base = pp * 130 + hh * 65
                wcol = pp * 128 + hh * 64
                wvTe[:, base:base + 64] = wvT_s[:, wcol:wcol + 64]
                bve[base:base + 64] = bv[sel][wcol:wcol + 64]
                bve[base + 64] = 1.0
        in_maps.append({
            "xT": xTb[b],
            "wqT": np.ascontiguousarray(Wq[sel].T).astype(bf16).reshape(NCHUNK, 128, 384),
            "wkT": np.ascontiguousarray(Wk[sel].T).astype(bf16).reshape(NCHUNK, 128, 384),
            "wvT": wvTe.astype(bf16).reshape(NCHUNK, 128, 390),
            "woT": WoT[sel].astype(bf16).reshape(NPAIR, 128, D),
            "bqv": np.ascontiguousarray(bq[sel].reshape(NPAIR, 128).T).astype(np.float32),
            "bkv": np.ascontiguousarray(bk[sel].reshape(NPAIR, 128).T).astype(np.float32),
            "bvv": bve.astype(bf16).reshape(1, 390),
            "uv": uvb[b],
            "wv": wvb[b],
        })
    return in_maps, perms


def combine_results(results, bo, perms):
    out = np.zeros((B, K, D), np.float32)
    for b in range(B):
        out[b][perms[b]] = (
            results[2 * b]["out"] + results[2 * b + 1]["out"] + bo
        )
    return out


def kernel(x, is_context, Wq, bq, Wk, bk, Wv, bv, Wo, bo):
    x = np.asarray(x, np.float32)
    is_context = np.asarray(is_context)
    args = [np.asarray(a, np.float32) for a in (Wq, bq, Wk, bk, Wv, bv, Wo)]
    sort = sort_ok(is_context)
    nc = _get_nc(sort)
    in_maps, perms = make_in_maps(x, is_context, *args, sort=sort)
    res = run_bass_kernel_spmd(nc, in_maps, list(range(N_CORES)), trace=False)
    return combine_results(res.results, np.asarray(bo, np.float32), perms)
